# revision 1
# baseline (speedup 1.0000x reference)
"""MHA kernel for TRN2: B=4,T=2048,D=1024,H=16,HD=64 across 8 NeuronCores.

Sharding: core c -> batch c//2, query half c%2 (host rotates the sequence so
each core's queries are rows 0:1024; softmax over keys is permutation
invariant). No collectives. All matmuls fp32r. Transposed-logits layout
(P^T [s,q]); a ones-column folded into V yields softmax denominators from the
same PV matmul; denominators are broadcast across partitions with a K=1 matmul.
"""
import sys
sys.path.insert(0, "/opt/trn_rl_repo")
import warnings
warnings.filterwarnings("ignore")

import numpy as np
import concourse.bass as bass
import concourse.mybir as mybir
import concourse.tile as tile
from concourse import bacc
from concourse.bass_utils import run_bass_kernel_spmd
from concourse.masks import make_identity

F32 = mybir.dt.float32
F32R = mybir.dt.float32r
EXP = mybir.ActivationFunctionType.Exp

T, D = 2048, 1024
TQ = 1024          # queries per core
NG = 8             # head groups (2 heads each)
NSC = 16           # s chunks of 128
NDC = 8            # d chunks of 128
SCALE = 0.125      # 1/sqrt(64)


def _transpose_8(nc, psw, nat, ident, copy_out):
    """Transpose nat [128,1024] in two 4-chunk batches; copy_out(half, psum_view)."""
    for half in range(2):
        p = psw.tile([128, 512], F32, tag="work")
        for k in range(4):
            dc = half * 4 + k
            nc.tensor.transpose(
                p[:, k * 128:(k + 1) * 128], nat[:, dc * 128:(dc + 1) * 128], ident)
        copy_out(half, p.rearrange("p (k f) -> p k f", k=4))


def build_nc():
    nc = bacc.Bacc("TRN2", target_bir_lowering=False, debug=False, num_devices=8)
    xin = nc.dram_tensor("xin", [T, D], F32, kind="ExternalInput")
    wq = nc.dram_tensor("wq", [D, D], F32, kind="ExternalInput")
    wk = nc.dram_tensor("wk", [D, D], F32, kind="ExternalInput")
    wv = nc.dram_tensor("wv", [D, D], F32, kind="ExternalInput")
    wo = nc.dram_tensor("wo", [D, D], F32, kind="ExternalInput")
    bo = nc.dram_tensor("bo", [1, D], F32, kind="ExternalInput")
    y = nc.dram_tensor("y", [TQ, D], F32, kind="ExternalOutput")

    with tile.TileContext(nc) as tc:
        with (
            tc.tile_pool(name="persist", bufs=1) as pp,
            tc.tile_pool(name="xtp", bufs=1) as xp,
            tc.tile_pool(name="vq", bufs=1) as vp,
            tc.tile_pool(name="wv1", bufs=1) as wvp,
            tc.tile_pool(name="work", bufs=2) as wp,
            tc.tile_pool(name="small", bufs=2) as sp,
            tc.tile_pool(name="ptp", bufs=3) as ptp,
            tc.tile_pool(name="ps_work", bufs=2, space="PSUM") as psw,
            tc.tile_pool(name="ps_pv", bufs=2, space="PSUM") as psv,
            tc.tile_pool(name="ps_log", bufs=2, space="PSUM") as psl,
        ):
            ident = pp.tile([128, 128], F32)
            make_identity(nc, ident)
            bias = pp.tile([128, D], F32)
            nc.sync.dma_start(
                out=bias, in_=bass.AP(tensor=bo, offset=0, ap=[[0, 128], [1, D]]))
            onesf = pp.tile([128, 64], F32)
            nc.vector.memset(onesf, 1.0)
            ones = pp.tile([65, 64], F32R)
            nc.vector.tensor_copy(out=ones, in_=onesf[0:65, :])
            catT = [pp.tile([128, TQ], F32R, tag=f"catT{g}", name=f"catT{g}")
                    for g in range(NG)]

            # ---- x^T : [128, dc, t] fp32r ----
            xT = xp.tile([128, NDC, T], F32R, tag="big")
            for tcb in range(NSC):
                nat = wp.tile([128, D], F32, tag="nat")
                nc.sync.dma_start(out=nat, in_=xin[tcb * 128:(tcb + 1) * 128, :])
                _transpose_8(nc, psw, nat, ident, lambda half, pv: nc.vector.tensor_copy(
                    out=xT[:, half * 4:half * 4 + 4, tcb * 128:(tcb + 1) * 128], in_=pv))

            vtile = None

            def build_vquarter(qid):
                """V for heads 4qid..4qid+3 -> [128 s, sc, 4 h, 65] (col 64 = ones)."""
                vt = vp.tile([128, NSC, 4, 65], F32R, tag="vq")
                wvT = wvp.tile([128, NDC, 256], F32R, tag="wvT")
                for rb in range(2):
                    nat = wp.tile([128, D], F32, tag="nat")
                    nc.sync.dma_start(
                        out=nat,
                        in_=wv[qid * 256 + rb * 128: qid * 256 + (rb + 1) * 128, :])
                    _transpose_8(nc, psw, nat, ident,
                                 lambda half, pv, rb=rb: nc.vector.tensor_copy(
                                     out=wvT[:, half * 4:half * 4 + 4,
                                             rb * 128:(rb + 1) * 128], in_=pv))
                for sc in range(NSC):
                    p = psw.tile([128, 512], F32, tag="work")
                    for dc in range(NDC):
                        nc.tensor.matmul(
                            p[:, 0:256], xT[:, dc, sc * 128:(sc + 1) * 128],
                            wvT[:, dc, :], start=(dc == 0), stop=(dc == NDC - 1))
                    nc.vector.tensor_copy(
                        out=vt[:, sc, :, 0:64],
                        in_=p[:, 0:256].rearrange("p (h c) -> p h c", h=4))
                nc.vector.tensor_copy(
                    out=vt[:, :, :, 64:65],
                    in_=onesf.rearrange("p (a b c) -> p a b c", a=NSC, b=4))
                return vt

            for g in range(NG):
                if g % 2 == 0:
                    vtile = build_vquarter(g // 2)
                i0 = 2 * (g % 2)  # head index within the quarter

                wqT = wp.tile([128, NDC, 128], F32R, tag="wqT")
                wkT = wp.tile([128, NDC, 128], F32R, tag="wkT")
                for (src, dst) in ((wq, wqT), (wk, wkT)):
                    nat = wp.tile([128, D], F32, tag="nat")
                    nc.sync.dma_start(out=nat, in_=src[g * 128:(g + 1) * 128, :])
                    _transpose_8(nc, psw, nat, ident,
                                 lambda half, pv, dst=dst: nc.vector.tensor_copy(
                                     out=dst[:, half * 4:half * 4 + 4, :], in_=pv))

                qT = wp.tile([128, TQ], F32R, tag="qT")
                for qh in range(2):
                    p = psw.tile([128, 512], F32, tag="work")
                    for dc in range(NDC):
                        nc.tensor.matmul(
                            p, wqT[:, dc, :], xT[:, dc, qh * 512:(qh + 1) * 512],
                            start=(dc == 0), stop=(dc == NDC - 1))
                    nc.vector.tensor_copy(out=qT[:, qh * 512:(qh + 1) * 512], in_=p)

                kT = wp.tile([128, T], F32R, tag="kT")
                for sb in range(4):
                    p = psw.tile([128, 512], F32, tag="work")
                    for dc in range(NDC):
                        nc.tensor.matmul(
                            p, wkT[:, dc, :], xT[:, dc, sb * 512:(sb + 1) * 512],
                            start=(dc == 0), stop=(dc == NDC - 1))
                    nc.vector.tensor_copy(out=kT[:, sb * 512:(sb + 1) * 512], in_=p)

                for qh in range(2):
                    qs = slice(qh * 512, (qh + 1) * 512)
                    pv0 = psv.tile([65, 512], F32, tag="pv")
                    pv1 = psv.tile([65, 512], F32, tag="pv")
                    for sc in range(NSC):
                        lg = psl.tile([128, 2, 512], F32, tag="log")
                        nc.tensor.matmul(
                            lg[:, 0, :], kT[0:64, sc * 128:(sc + 1) * 128],
                            qT[0:64, qs], start=True, stop=True)
                        nc.tensor.matmul(
                            lg[:, 1, :], kT[64:128, sc * 128:(sc + 1) * 128],
                            qT[64:128, qs], start=True, stop=True)
                        pt = ptp.tile([128, 2, 512], F32R, tag="pt")
                        nc.scalar.activation(
                            out=pt.rearrange("p a b -> p (a b)"),
                            in_=lg.rearrange("p a b -> p (a b)"),
                            func=EXP, scale=SCALE)
                        nc.tensor.matmul(
                            pv0, vtile[:, sc, i0, :], pt[:, 0, :],
                            start=(sc == 0), stop=(sc == NSC - 1))
                        nc.tensor.matmul(
                            pv1, vtile[:, sc, i0 + 1, :], pt[:, 1, :],
                            start=(sc == 0), stop=(sc == NSC - 1))
                    for hloc, pv in ((0, pv0), (1, pv1)):
                        # sums (row 64) -> broadcast to 64 partitions via K=1 matmul
                        s1 = sp.tile([65, 512], F32R, tag="s1")
                        nc.vector.tensor_copy(out=s1[64:65, :], in_=pv[64:65, :])
                        pb = psw.tile([128, 512], F32, tag="work")
                        nc.tensor.matmul(
                            pb[0:64, :], ones[64:65, :], s1[64:65, :],
                            start=True, stop=True)
                        rec = sp.tile([64, 512], F32, tag="rec")
                        nc.vector.reciprocal(out=rec, in_=pb[0:64, :])
                        if hloc == 0:
                            nc.vector.tensor_mul(
                                out=catT[g][0:64, qs], in0=pv[0:64, :], in1=rec)
                        else:
                            tmp = sp.tile([64, 512], F32R, tag="tmp")
                            nc.vector.tensor_mul(out=tmp, in0=pv[0:64, :], in1=rec)
                            nc.sync.dma_start(out=catT[g][64:128, qs], in_=tmp)

            # ---- final projection (woT reuses xT's slot) ----
            woT = xp.tile([128, NDC, D], F32R, tag="big")
            for rb in range(NDC):
                nat = wp.tile([128, D], F32, tag="nat")
                nc.sync.dma_start(out=nat, in_=wo[rb * 128:(rb + 1) * 128, :])
                _transpose_8(nc, psw, nat, ident,
                             lambda half, pv, rb=rb: nc.vector.tensor_copy(
                                 out=woT[:, half * 4:half * 4 + 4,
                                         rb * 128:(rb + 1) * 128], in_=pv))
            for qb in range(8):
                yt = wp.tile([128, D], F32, tag="yt")
                for nh in range(2):
                    p = psw.tile([128, 512], F32, tag="work")
                    for g in range(NG):
                        nc.tensor.matmul(
                            p, catT[g][:, qb * 128:(qb + 1) * 128],
                            woT[:, g, nh * 512:(nh + 1) * 512],
                            start=(g == 0), stop=(g == NG - 1))
                    nc.vector.tensor_add(
                        out=yt[:, nh * 512:(nh + 1) * 512], in0=p,
                        in1=bias[:, nh * 512:(nh + 1) * 512])
                nc.sync.dma_start(out=y[qb * 128:(qb + 1) * 128, :], in_=yt)

    nc.compile()
    return nc


_CACHE = {}


def kernel(x, Wq, Wk, Wv, Wo, bo):
    if "nc" not in _CACHE:
        _CACHE["nc"] = build_nc()
    nc = _CACHE["nc"]
    x = np.ascontiguousarray(x, dtype=np.float32)
    wq2 = np.ascontiguousarray(Wq.reshape(D, D), dtype=np.float32)
    wk2 = np.ascontiguousarray(Wk.reshape(D, D), dtype=np.float32)
    wv2 = np.ascontiguousarray(Wv.reshape(D, D), dtype=np.float32)
    wo2 = np.ascontiguousarray(Wo, dtype=np.float32)
    bo2 = np.ascontiguousarray(bo.reshape(1, D), dtype=np.float32)
    in_maps = []
    for c in range(8):
        b, h = c // 2, c % 2
        xin = x[b] if h == 0 else np.concatenate([x[b, TQ:], x[b, :TQ]], axis=0)
        in_maps.append({"xin": np.ascontiguousarray(xin), "wq": wq2, "wk": wk2,
                        "wv": wv2, "wo": wo2, "bo": bo2})
    res = run_bass_kernel_spmd(nc, in_maps, core_ids=list(range(8)))
    out = np.empty((4, T, D), dtype=np.float32)
    for c in range(8):
        b, h = c // 2, c % 2
        out[b, h * TQ:(h + 1) * TQ] = res.results[c]["y"]
    return out

